# revision 24
# baseline (speedup 1.0000x reference)
"""Catmull-Rom 4D spline interpolation kernel for Trainium2 (8 NeuronCores).

Problem: knots [16,64,128,128,2] f32, idx [262144,3] f32 (z,y,x coords),
depth scalar -> out [262144, 2] f32.

Strategy (v3, fp16 + dma_gather):
  - depth is a scalar -> D collapses host-side to 4 slabs + weights wd.
  - Points sharded by fixed z-cell runs ([8,8,8,8,7,7,7,7] over iz in
    [1,60]), so each core spans <= 8 z-cells -> its folded table has
    8*125*32 = 32000 quad-rows, indexable by int16 dma_gather indices.
  - Phase A (per core): depth-reduce the 11-slab fp16 z-window to
    V12[y,z,c,x]; fold the y-spline basis (jy-expansion via shifted
    partition copies) then the z-spline basis (jz-expansion via shared
    difference tensors E[z]=A[z]-A[z+1]) producing W3 rows
    (az, ay, axq) -> [c, jz, jy, ax4] = 128 fp16 = 256 B (quad-packed x).
  - Phase B: one dma_gather descriptor per point reads 512 B (2 quad-rows
    = 8 ax slots covering the point's 4-ax window at quad offset q);
    multiply by host-shipped cardinal x-weights cxw8 (zeros outside the
    window) and polynomial wzy = sz^jz * sy^jy, then tree-reduce. All
    multiplies run in DVE 2x fp16 mode (packed last axis).
"""
import sys

sys.path.insert(0, "/opt/trn_rl_repo")

import numpy as np

import concourse.mybir as mybir
import concourse.tile as tile_mod
from concourse import bass
from concourse.bacc import Bacc
from concourse.tile import TileContext
from concourse import bass_utils, library_config

# ---------------------------------------------------------------------------
# Workaround: this walrus build allows 1 sync wait per instruction (2 on
# InstEventSemaphore), but TileContext's tail drain carries one wait per DMA
# sem lane. Split the drain's waits onto EventSemaphore instructions.


def _patched_dab(self, tick_clock, wait_clock):
    nc = self.nc
    drain_bi = nc.sync.drain()
    wait_clock.add_sem_waits(
        drain_bi.ins, tile_mod.ScopedClock({None: tick_clock.global_clock})
    )
    si = drain_bi.ins.sync_info
    waits = list(si.on_wait) if si is not None else []
    if len(waits) > 1:
        si.on_wait = []
        bb = nc.cur_bb.bb
        insts = bb.instructions
        assert insts[-1].name == drain_bi.ins.name
        insts.pop()
        for i in range(0, len(waits), 2):
            ev = mybir.InstEventSemaphore(
                name=nc.get_next_instruction_name(), ins=[], outs=[]
            )
            ev.engine = drain_bi.ins.engine
            ev.sync_info = mybir.SyncInfo(on_wait=waits[i : i + 2], on_update=[])
            nc.register_instruction(ev)
            bb.add_instruction(ev)
        bb.add_instruction(drain_bi.ins)
    nc.all_engine_barrier()
    assert self.sems is not None
    popped = nc._tile_sem_poison_stack.pop()
    assert popped is self._sem_poison
    nc.clear_and_free_semaphores(list(self.sems.allocated().values()))
    nc.all_engine_barrier()


tile_mod.TileContext._drain_and_barrier = _patched_dab

# ---------------------------------------------------------------------------
D, Z, Y, X, C = 16, 64, 128, 128, 2
N = 262144
NCORES = 8
P = 128

ZRUNS = [8, 8, 8, 8, 7, 7, 7, 7]  # z-cells per core, covering iz in [1,60]
ZW = 11  # z-slab window per core (max run 8 + 3)
AZT = 8  # az table extent per core
NAY = 125  # ay in [0,124]
AXQ = 32  # x quads
NROWS = AZT * NAY * AXQ  # 32000 (+1 pad row)
NPC = 35840  # padded points per core
NB = NPC // P  # 280 blocks
GI = 1024  # idxs per dma_gather call (hw limit ~1024)
NG = NPC // GI  # 35 gather calls
GB = GI // P  # 8 blocks per gather
NCH = 7  # compute chunks
CB = NB // NCH  # 40 blocks per chunk
CG = NG // NCH  # 5 gathers per chunk

f32 = mybir.dt.float32
fp16 = mybir.dt.float16
i16 = mybir.dt.int16
AluOp = mybir.AluOpType

_HERMITE = np.array(
    [[2, -2, 1, 1], [-3, 3, -2, -1], [0, 0, 1, 0], [1, 0, 0, 0]], dtype=np.float64
)
_CR = np.array(
    [[0, 1, 0, 0], [0, 0, 1, 0], [-0.5, 0, 0.5, 0], [0, -0.5, 0, 0.5]],
    dtype=np.float64,
)
BASIS = _HERMITE @ _CR  # [4 powers (s^3..s^0), 4 knots]
BB = BASIS[::-1].copy()  # rows s^0..s^3 (jy/jz coefficient of each knot)


def build_kernel(reps=1, phases="AB"):
    """Per-core kernel (SPMD; per-core data differs). Inputs:
    knots11 [4, ZW, Y, C, X] fp16   host-sliced depth+z window (c before x)
    wd      [P, 4] f32              depth weights (replicated over partitions)
    idxs16  [128, NG*GI/16] i16     wrapped+replicated gather indices
    wb      [P, NB, 24] fp16        per-point cxw8 (8) + wzy (16)
    Output: out [P, NB*2] f32
    """
    nc = Bacc(
        "TRN2",
        target_bir_lowering=False,
        debug=False,
        num_devices=NCORES,
        num_swdge_queues=2,
    )
    knots11 = nc.dram_tensor("knots11", [4, ZW, Y, C, X], fp16, kind="ExternalInput")
    wd = nc.dram_tensor("wd", [P, 4], f32, kind="ExternalInput")
    idxs16 = nc.dram_tensor("idxs16", [128, NG * GI // 16], i16, kind="ExternalInput")
    wx8 = nc.dram_tensor("wx8", [P, NB * 8], fp16, kind="ExternalInput")
    wzy16 = nc.dram_tensor("wzy16", [P, NB * 16], fp16, kind="ExternalInput")
    out = nc.dram_tensor("out", [P, NB * 2], f32, kind="ExternalOutput")
    w3rows = nc.dram_tensor("w3rows", [NROWS + 1, 128], fp16, kind="Internal")

    with TileContext(nc) as tc:
      if "B" in phases:
          nc.gpsimd.load_library(library_config.mlp)
      for _rep in range(reps):
        with tc.tile_pool(name="const", bufs=1) as cpool:
            wd_sb = cpool.tile([P, 4], f32)
            nc.sync.dma_start(out=wd_sb[:], in_=wd[:])
            idx_sb = cpool.tile([128, NG * GI // 16], i16)
            nc.sync.dma_start(out=idx_sb[:], in_=idxs16[:])
            wx_sb = cpool.tile([P, NB, 8], fp16)
            nc.sync.dma_start(out=wx_sb[:].rearrange("p b w -> p (b w)"), in_=wx8[:])
            wzy_sb = cpool.tile([P, NB, 16], fp16)
            nc.sync.dma_start(
                out=wzy_sb[:].rearrange("p b w -> p (b w)"), in_=wzy16[:]
            )

            if "A" in phases:
                # V12[y, z, c, x] fp16: depth-reduced window
                v12 = cpool.tile([P, ZW, C, X], fp16)
                with tc.tile_pool(name="pa", bufs=2) as pa:
                    for z0, zn in [(0, 4), (4, 4), (8, 3)]:
                        slabs = pa.tile([P, 4, zn, C * X], fp16, tag="slabs")
                        for d in range(4):
                            nc.sync.dma_start(
                                out=slabs[:, d, :, :],
                                in_=knots11[d, z0 : z0 + zn, :, :, :].rearrange(
                                    "z y c x -> y z (c x)"
                                ),
                            )
                        vsl = v12[:, z0 : z0 + zn, :, :].rearrange(
                            "p z c x -> p (z c x)"
                        )
                        nc.vector.tensor_scalar(
                            out=vsl,
                            in0=slabs[:, 0, :, :].rearrange("p z f -> p (z f)"),
                            scalar1=wd_sb[:, 0:1],
                            scalar2=None,
                            op0=AluOp.mult,
                        )
                        for d in range(1, 4):
                            nc.vector.scalar_tensor_tensor(
                                out=vsl,
                                in0=slabs[:, d, :, :].rearrange("p z f -> p (z f)"),
                                scalar=wd_sb[:, d : d + 1],
                                in1=vsl,
                                op0=AluOp.mult,
                                op1=AluOp.add,
                            )

                # ky-shifted copies of V12 (partition shifts via SBUF DMA)
                v12s = [v12]
                for ky in range(1, 4):
                    vk = cpool.tile([P, ZW, C, X], fp16, tag=f"v12s{ky}")
                    nc.sync.dma_start(
                        out=vk[0 : P - ky, :, :, :], in_=v12[ky:P, :, :, :]
                    )
                    v12s.append(vk)

                # W3 row (az, ay, axq) payload [c, kz, ky, ax4]: raw z/y
                # neighborhood values (cardinal basis; weights ship from host)
                with tc.tile_pool(name="pc", bufs=2) as pc:
                    for az in range(AZT):
                        w3t = pc.tile([P, AXQ, C, 4, 4, 4], fp16, tag="w3t")
                        for ky in range(4):
                            for c in range(C):
                                # <=3 free dims per ISA operand, shapes match
                                nc.vector.tensor_copy(
                                    out=w3t[0:NAY, :, c, :, ky, :],
                                    in_=v12s[ky][
                                        0:NAY, az : az + 4, c, :
                                    ].rearrange(
                                        "p kz (axq ax4) -> p axq kz ax4",
                                        axq=AXQ, ax4=4,
                                    ),
                                )
                        nc.sync.dma_start(
                            out=w3rows[az * NAY * AXQ : (az + 1) * NAY * AXQ, :]
                            .rearrange("(ay axq) f -> ay (axq f)", ay=NAY, axq=AXQ),
                            in_=w3t[0:NAY].rearrange(
                                "p axq c jz jy ax4 -> p (axq c jz jy ax4)"
                            ),
                        )
                # zero the pad row (read by idx NROWS-1 overlap)
                zt = cpool.tile([P, 128], fp16, tag="zt")
                nc.vector.memset(zt[0:1, :], 0.0)
                nc.sync.dma_start(out=w3rows[NROWS : NROWS + 1, :], in_=zt[0:1, :])

            if "B" in phases:
                with tc.tile_pool(name="pb", bufs=3) as pb:
                    for ch in range(NCH):
                        g = pb.tile([P, CB, 256], fp16, tag="g")
                        for ci in range(CG):
                            gc = ch * CG + ci
                            nc.gpsimd.dma_gather(
                                out_ap=g[:, ci * GB : (ci + 1) * GB, :],
                                in_ap=bass.AP(w3rows, 0, [[128, NROWS], [1, 256]]),
                                idxs_ap=idx_sb[
                                    :, gc * (GI // 16) : (gc + 1) * (GI // 16)
                                ],
                                num_idxs=GI,
                                num_idxs_reg=GI,
                                elem_size=256,
                                elem_step=128,
                                queue_num=gc % 2,
                            )
                        # g: [p, (b r), m=(c kz ky), ax4] (<=3 free dims)
                        gv = g[:].rearrange(
                            "p b (r m ax) -> p (b r) m ax", r=2, m=32, ax=4
                        )
                        # p1: g *= cxw8 (bcast over m)
                        cxwb = (
                            wx_sb[:, ch * CB : (ch + 1) * CB, :]
                            .rearrange("p b (r i ax) -> p (b r) i ax", r=2, i=1, ax=4)
                            .to_broadcast([P, CB * 2, 32, 4])
                        )
                        nc.vector.tensor_tensor(
                            out=gv, in0=gv, in1=cxwb, op=AluOp.mult
                        )
                        # fold row2
                        g2 = g[:].rearrange("p b (r f) -> p b r f", r=2, f=128)
                        t = pb.tile([P, CB, 32, 4], fp16, tag="t")
                        nc.vector.tensor_tensor(
                            out=t[:].rearrange("p b m ax -> p b (m ax)"),
                            in0=g2[:, :, 0],
                            in1=g2[:, :, 1],
                            op=AluOp.add,
                        )
                        # fold ax4 4->2->1
                        u = pb.tile([P, CB, 32, 2], fp16, tag="u")
                        nc.vector.tensor_tensor(
                            out=u[:], in0=t[:, :, :, 0:2], in1=t[:, :, :, 2:4],
                            op=AluOp.add,
                        )
                        v = pb.tile([P, CB, 2, 16], fp16, tag="v")
                        vf = v[:].rearrange("p b c k -> p b (c k)")
                        nc.vector.tensor_tensor(
                            out=vf, in0=u[:, :, :, 0], in1=u[:, :, :, 1],
                            op=AluOp.add,
                        )
                        # *= wzy (bcast over c)
                        wzyb = (
                            wzy_sb[:, ch * CB : (ch + 1) * CB, :]
                            .rearrange("p b (i k) -> p b i k", i=1, k=16)
                            .to_broadcast([P, CB, 2, 16])
                        )
                        nc.vector.tensor_tensor(
                            out=v[:], in0=v[:], in1=wzyb, op=AluOp.mult
                        )
                        # reduce (kz, ky) -> f32
                        ov = pb.tile([P, CB, 2], f32, tag="ov")
                        nc.vector.tensor_reduce(
                            out=ov[:],
                            in_=v[:],
                            axis=mybir.AxisListType.X,
                            op=AluOp.add,
                        )
                        nc.sync.dma_start(
                            out=out[:, ch * CB * 2 : (ch + 1) * CB * 2],
                            in_=ov[:].rearrange("p b c -> p (b c)"),
                        )
            elif "A" in phases:
                zo = cpool.tile([P, NB * 2], f32, tag="zo")
                nc.vector.memset(zo[:], 0.0)
                nc.sync.dma_start(out=out[:], in_=zo[:])
    nc.compile()
    return nc


# ---------------------------------------------------------------------------
_BUILT = None


def _get_built():
    global _BUILT
    if _BUILT is None:
        _BUILT = build_kernel()
    return _BUILT


def _host_prep(idx, knots, depth):
    depth = float(depth)
    ind = int(
        np.searchsorted(np.arange(1, D + 1, dtype=np.float64), depth, side="right")
    )
    ind = max(1, min(ind, D - 1))
    r = depth - float(ind)
    dcoord = (ind - 1) + r
    i0 = int(np.floor(dcoord))
    sd = dcoord - i0
    idp = np.clip(i0 - 1 + np.arange(4), 0, D - 1)
    powers = np.array([sd**3, sd**2, sd, 1.0], dtype=np.float64)
    wdv = (powers @ BASIS).astype(np.float32)
    wd_rep = np.tile(wdv[None, :], (P, 1))
    knots4 = knots[idp]  # [4, Z, Y, X, C] f32 view

    co = idx.astype(np.float64)
    iz = np.floor(co[:, 0]).astype(np.int64)
    iy = np.floor(co[:, 1]).astype(np.int64)
    ix = np.floor(co[:, 2]).astype(np.int64)
    sz = co[:, 0] - iz
    sy = co[:, 1] - iy
    sx = co[:, 2] - ix

    # x-window cardinal weights over 8 quad slots
    cx4 = (
        np.stack([sx**3, sx**2, sx, np.ones_like(sx)], 1) @ BASIS
    )  # [N, 4]
    q = ((ix - 1) & 3).astype(np.int64)
    cxw8 = np.zeros((N, 8), np.float64)
    np.put_along_axis(cxw8, q[:, None] + np.arange(4)[None, :], cx4, axis=1)
    cz4 = np.stack([sz**3, sz**2, sz, np.ones_like(sz)], 1) @ BASIS
    cy4 = np.stack([sy**3, sy**2, sy, np.ones_like(sy)], 1) @ BASIS
    wzy_all = (
        (cz4[:, :, None] * cy4[:, None, :]).reshape(N, 16).astype(np.float16)
    )
    wx_all = cxw8.astype(np.float16)

    in_maps = []
    unpack = []
    c0 = 1
    for core in range(NCORES):
        run = ZRUNS[core]
        sel = np.where((iz >= c0) & (iz < c0 + run))[0]
        n = len(sel)
        assert n <= NPC, (core, n)
        az = iz[sel] - c0
        row = ((az * NAY + (iy[sel] - 1)) * AXQ + ((ix[sel] - 1) >> 2)).astype(
            np.int64
        )
        order = np.argsort(row, kind="stable")
        sel = sel[order]
        rows_s = row[order]
        rows_pad = np.concatenate(
            [rows_s, np.full(NPC - n, rows_s[0] if n else 0, np.int64)]
        ).astype(np.int16)
        wx_pad = np.zeros((NPC, 8), np.float16)
        wx_pad[:n] = wx_all[sel]
        wzy_pad = np.zeros((NPC, 16), np.float16)
        wzy_pad[:n] = wzy_all[sel]

        # idxs: per 1024-call wrapped [16, 64], replicated to 128 partitions
        blk = (
            rows_pad.reshape(NG, 64, 16).transpose(0, 2, 1).reshape(NG, 16, 64)
        )  # [NG, 16, 64]
        idxs_core = np.tile(
            blk.transpose(1, 0, 2).reshape(16, NG * 64), (8, 1)
        )  # [128, NG*64]

        # weights: j = call*1024 + bl*128 + p -> [p, call*8 + bl]
        wx_core = (
            wx_pad.reshape(NG, GB, P, 8).transpose(2, 0, 1, 3).reshape(P, NB * 8)
        )
        wzy_core = (
            wzy_pad.reshape(NG, GB, P, 16).transpose(2, 0, 1, 3).reshape(P, NB * 16)
        )

        zs = c0 - 1
        kn = np.ascontiguousarray(
            knots4[:, zs : zs + ZW].transpose(0, 1, 2, 4, 3)
        ).astype(np.float16)  # [4, ZW, Y, C, X]

        in_maps.append(
            {
                "knots11": kn,
                "wd": wd_rep,
                "idxs16": np.ascontiguousarray(idxs_core),
                "wx8": np.ascontiguousarray(wx_core),
                "wzy16": np.ascontiguousarray(wzy_core),
            }
        )
        unpack.append((sel, n))
        c0 += run
    return in_maps, unpack


def kernel(idx, knots, depth):
    idx = np.asarray(idx, dtype=np.float32)
    knots = np.asarray(knots, dtype=np.float32)
    nc = _get_built()
    in_maps, unpack = _host_prep(idx, knots, depth)
    res = bass_utils.run_bass_kernel_spmd(nc, in_maps, core_ids=list(range(NCORES)))
    out_full = np.empty((N, 2), np.float32)
    for core in range(NCORES):
        sel, n = unpack[core]
        o = (
            res.results[core]["out"]
            .reshape(P, NG, GB, 2)
            .transpose(1, 2, 0, 3)
            .reshape(NPC, 2)
        )
        out_full[sel] = o[:n]
    return out_full


if __name__ == "__main__":
    nc = build_kernel()
    print("built ok")


# revision 27
# speedup vs baseline: 2.0070x; 2.0070x over previous
"""Catmull-Rom 4D spline interpolation kernel for Trainium2 (8 NeuronCores).

Problem: knots [16,64,128,128,2] f32, idx [262144,3] f32 (z,y,x coords),
depth scalar -> out [262144, 2] f32.

Strategy (v3, fp16 + dma_gather):
  - depth is a scalar -> D collapses host-side to 4 slabs + weights wd.
  - Points sharded by fixed z-cell runs ([8,8,8,8,7,7,7,7] over iz in
    [1,60]), so each core spans <= 8 z-cells -> its folded table has
    8*125*32 = 32000 quad-rows, indexable by int16 dma_gather indices.
  - Phase A (per core): depth-reduce the 11-slab fp16 z-window to
    V12[y,z,c,x], then lay out W3 rows (az, ay, axq) -> [c, kz, ky, ax4]
    = 128 fp16 = 256 B: the raw 4x4 z/y neighborhood per x-quad (cardinal
    basis -- pure copies from ky-shifted V12, no basis expansion; the
    spline weights ship from the host instead).
  - Phase B: one dma_gather descriptor per point reads 512 B (2 quad-rows
    = 8 ax slots covering the point's 4-ax window at quad offset q);
    multiply by host-shipped cardinal x-weights cxw8 (zeros outside the
    window) and wzy = wz4 (x) wy4 Catmull weights, then tree-reduce.
    The main multiplies run in DVE 2x fp16 mode (packed last axis).
"""
import sys

sys.path.insert(0, "/opt/trn_rl_repo")

import numpy as np

import concourse.mybir as mybir
import concourse.tile as tile_mod
from concourse import bass
from concourse.bacc import Bacc
from concourse.tile import TileContext
from concourse import bass_utils, library_config

# ---------------------------------------------------------------------------
# Workaround: this walrus build allows 1 sync wait per instruction (2 on
# InstEventSemaphore), but TileContext's tail drain carries one wait per DMA
# sem lane. Split the drain's waits onto EventSemaphore instructions.


def _patched_dab(self, tick_clock, wait_clock):
    nc = self.nc
    drain_bi = nc.sync.drain()
    wait_clock.add_sem_waits(
        drain_bi.ins, tile_mod.ScopedClock({None: tick_clock.global_clock})
    )
    si = drain_bi.ins.sync_info
    waits = list(si.on_wait) if si is not None else []
    if len(waits) > 1:
        si.on_wait = []
        bb = nc.cur_bb.bb
        insts = bb.instructions
        assert insts[-1].name == drain_bi.ins.name
        insts.pop()
        for i in range(0, len(waits), 2):
            ev = mybir.InstEventSemaphore(
                name=nc.get_next_instruction_name(), ins=[], outs=[]
            )
            ev.engine = drain_bi.ins.engine
            ev.sync_info = mybir.SyncInfo(on_wait=waits[i : i + 2], on_update=[])
            nc.register_instruction(ev)
            bb.add_instruction(ev)
        bb.add_instruction(drain_bi.ins)
    nc.all_engine_barrier()
    assert self.sems is not None
    popped = nc._tile_sem_poison_stack.pop()
    assert popped is self._sem_poison
    nc.clear_and_free_semaphores(list(self.sems.allocated().values()))
    nc.all_engine_barrier()


tile_mod.TileContext._drain_and_barrier = _patched_dab

# ---------------------------------------------------------------------------
D, Z, Y, X, C = 16, 64, 128, 128, 2
N = 262144
NCORES = 8
P = 128

ZRUNS = [8, 8, 8, 8, 7, 7, 7, 7]  # z-cells per core, covering iz in [1,60]
ZW = 11  # z-slab window per core (max run 8 + 3)
AZT = 8  # az table extent per core
NAY = 125  # ay in [0,124]
AXQ = 32  # x quads
NROWS = AZT * NAY * AXQ  # 32000 (+1 pad row)
NPC = 35840  # padded points per core
NB = NPC // P  # 280 blocks
GI = 1024  # idxs per dma_gather call (hw limit ~1024)
NG = NPC // GI  # 35 gather calls
GB = GI // P  # 8 blocks per gather
NCH = 7  # compute chunks
CB = NB // NCH  # 40 blocks per chunk
CG = NG // NCH  # 5 gathers per chunk

f32 = mybir.dt.float32
fp16 = mybir.dt.float16
i16 = mybir.dt.int16
AluOp = mybir.AluOpType

_HERMITE = np.array(
    [[2, -2, 1, 1], [-3, 3, -2, -1], [0, 0, 1, 0], [1, 0, 0, 0]], dtype=np.float64
)
_CR = np.array(
    [[0, 1, 0, 0], [0, 0, 1, 0], [-0.5, 0, 0.5, 0], [0, -0.5, 0, 0.5]],
    dtype=np.float64,
)
BASIS = _HERMITE @ _CR  # [4 powers (s^3..s^0), 4 knots]
BB = BASIS[::-1].copy()  # rows s^0..s^3 (jy/jz coefficient of each knot)


def build_kernel(reps=1, phases="AB"):
    """Per-core kernel (SPMD; per-core data differs). Inputs:
    knots11 [4, ZW, Y, C, X] fp16   host-sliced depth+z window (c before x)
    wd      [P, 4] f32              depth weights (replicated over partitions)
    idxs16  [128, NG*GI/16] i16     wrapped+replicated gather indices
    wb      [P, NB, 24] fp16        per-point cxw8 (8) + wzy (16)
    Output: out [P, NB*2] f32
    """
    nc = Bacc("TRN2", target_bir_lowering=False, debug=False, num_devices=NCORES)
    knots11 = nc.dram_tensor("knots11", [4, ZW, Y, C, X], fp16, kind="ExternalInput")
    wd = nc.dram_tensor("wd", [P, 4], f32, kind="ExternalInput")
    idxs16 = nc.dram_tensor("idxs16", [128, NG * GI // 16], i16, kind="ExternalInput")
    wx8 = nc.dram_tensor("wx8", [P, NB * 8], fp16, kind="ExternalInput")
    wzy16 = nc.dram_tensor("wzy16", [P, NB * 16], fp16, kind="ExternalInput")
    out = nc.dram_tensor("out", [P, NB * 2], f32, kind="ExternalOutput")
    w3rows = nc.dram_tensor("w3rows", [NROWS + 1, 128], fp16, kind="Internal")

    with TileContext(nc) as tc:
      for _rep in range(reps):
        with tc.tile_pool(name="const", bufs=1) as cpool:
            wd_sb = cpool.tile([P, 4], f32)
            nc.sync.dma_start(out=wd_sb[:], in_=wd[:])
            idx_sb = cpool.tile([128, NG * GI // 16], i16)
            nc.sync.dma_start(out=idx_sb[:], in_=idxs16[:])
            wx_sb = cpool.tile([P, NB, 8], fp16)
            nc.sync.dma_start(out=wx_sb[:].rearrange("p b w -> p (b w)"), in_=wx8[:])
            wzy_sb = cpool.tile([P, NB, 16], fp16)
            nc.sync.dma_start(
                out=wzy_sb[:].rearrange("p b w -> p (b w)"), in_=wzy16[:]
            )

            if "A" in phases:
                # V12[y, z, c, x] fp16: depth-reduced window
                v12 = cpool.tile([P, ZW, C, X], fp16)
                with tc.tile_pool(name="pa", bufs=2) as pa:
                    for z0, zn in [(0, 4), (4, 4), (8, 3)]:
                        slabs = pa.tile([P, 4, zn, C * X], fp16, tag="slabs")
                        for d in range(4):
                            nc.sync.dma_start(
                                out=slabs[:, d, :, :],
                                in_=knots11[d, z0 : z0 + zn, :, :, :].rearrange(
                                    "z y c x -> y z (c x)"
                                ),
                            )
                        vsl = v12[:, z0 : z0 + zn, :, :].rearrange(
                            "p z c x -> p (z c x)"
                        )
                        nc.vector.tensor_scalar(
                            out=vsl,
                            in0=slabs[:, 0, :, :].rearrange("p z f -> p (z f)"),
                            scalar1=wd_sb[:, 0:1],
                            scalar2=None,
                            op0=AluOp.mult,
                        )
                        for d in range(1, 4):
                            nc.vector.scalar_tensor_tensor(
                                out=vsl,
                                in0=slabs[:, d, :, :].rearrange("p z f -> p (z f)"),
                                scalar=wd_sb[:, d : d + 1],
                                in1=vsl,
                                op0=AluOp.mult,
                                op1=AluOp.add,
                            )

                # ky-shifted copies of V12 (partition shifts via SBUF DMA)
                v12s = [v12]
                for ky in range(1, 4):
                    vk = cpool.tile([P, ZW, C, X], fp16, tag=f"v12s{ky}")
                    nc.sync.dma_start(
                        out=vk[0 : P - ky, :, :, :], in_=v12[ky:P, :, :, :]
                    )
                    v12s.append(vk)

                # W3 row (az, ay, axq) payload [c, kz, ky, ax4]: raw z/y
                # neighborhood values (cardinal basis; weights ship from host)
                with tc.tile_pool(name="pc", bufs=2) as pc:
                    for az in range(AZT):
                        w3t = pc.tile([P, AXQ, C, 4, 4, 4], fp16, tag="w3t")
                        for ky in range(4):
                            for c in range(C):
                                # <=3 free dims per ISA operand, shapes match
                                nc.vector.tensor_copy(
                                    out=w3t[0:NAY, :, c, :, ky, :],
                                    in_=v12s[ky][
                                        0:NAY, az : az + 4, c, :
                                    ].rearrange(
                                        "p kz (axq ax4) -> p axq kz ax4",
                                        axq=AXQ, ax4=4,
                                    ),
                                )
                        nc.sync.dma_start(
                            out=w3rows[az * NAY * AXQ : (az + 1) * NAY * AXQ, :]
                            .rearrange("(ay axq) f -> ay (axq f)", ay=NAY, axq=AXQ),
                            in_=w3t[0:NAY].rearrange(
                                "p axq c jz jy ax4 -> p (axq c jz jy ax4)"
                            ),
                        )
                # zero the pad row (read by idx NROWS-1 overlap)
                zt = cpool.tile([P, 128], fp16, tag="zt")
                nc.vector.memset(zt[0:1, :], 0.0)
                nc.sync.dma_start(out=w3rows[NROWS : NROWS + 1, :], in_=zt[0:1, :])

            if "B" in phases:
                nc.gpsimd.load_library(library_config.mlp)
                # per-gather tiles: each dma_gather owns its tile so the Pool
                # descriptor-gen of gather i+1 overlaps the DVE chain of
                # gather i (no whole-chunk tile to serialize on).
                with tc.tile_pool(name="pb", bufs=4) as pb:
                    for gc in range(NG):
                        g = pb.tile([P, GB, 256], fp16, tag="g")
                        nc.gpsimd.dma_gather(
                            out_ap=g[:],
                            in_ap=bass.AP(w3rows, 0, [[128, NROWS], [1, 256]]),
                            idxs_ap=idx_sb[
                                :, gc * (GI // 16) : (gc + 1) * (GI // 16)
                            ],
                            num_idxs=GI,
                            num_idxs_reg=GI,
                            elem_size=256,
                            elem_step=128,
                        )
                        # g: [p, (b r), m=(c kz ky), ax4] (<=3 free dims)
                        gv = g[:].rearrange(
                            "p b (r m ax) -> p (b r) m ax", r=2, m=32, ax=4
                        )
                        # p1: g *= cxw8 (bcast over m)
                        cxwb = (
                            wx_sb[:, gc * GB : (gc + 1) * GB, :]
                            .rearrange("p b (r i ax) -> p (b r) i ax", r=2, i=1, ax=4)
                            .to_broadcast([P, GB * 2, 32, 4])
                        )
                        nc.vector.tensor_tensor(
                            out=gv, in0=gv, in1=cxwb, op=AluOp.mult
                        )
                        # fold row2
                        g2 = g[:].rearrange("p b (r f) -> p b r f", r=2, f=128)
                        t = pb.tile([P, GB, 32, 4], fp16, tag="t")
                        nc.vector.tensor_tensor(
                            out=t[:].rearrange("p b m ax -> p b (m ax)"),
                            in0=g2[:, :, 0],
                            in1=g2[:, :, 1],
                            op=AluOp.add,
                        )
                        # fold ax4 4->2->1
                        u = pb.tile([P, GB, 32, 2], fp16, tag="u")
                        nc.vector.tensor_tensor(
                            out=u[:], in0=t[:, :, :, 0:2], in1=t[:, :, :, 2:4],
                            op=AluOp.add,
                        )
                        v = pb.tile([P, GB, 2, 16], fp16, tag="v")
                        vf = v[:].rearrange("p b c k -> p b (c k)")
                        nc.vector.tensor_tensor(
                            out=vf, in0=u[:, :, :, 0], in1=u[:, :, :, 1],
                            op=AluOp.add,
                        )
                        # *= wzy (bcast over c)
                        wzyb = (
                            wzy_sb[:, gc * GB : (gc + 1) * GB, :]
                            .rearrange("p b (i k) -> p b i k", i=1, k=16)
                            .to_broadcast([P, GB, 2, 16])
                        )
                        nc.vector.tensor_tensor(
                            out=v[:], in0=v[:], in1=wzyb, op=AluOp.mult
                        )
                        # reduce (kz, ky) -> f32
                        ov = pb.tile([P, GB, 2], f32, tag="ov")
                        nc.vector.tensor_reduce(
                            out=ov[:],
                            in_=v[:],
                            axis=mybir.AxisListType.X,
                            op=AluOp.add,
                        )
                        nc.sync.dma_start(
                            out=out[:, gc * GB * 2 : (gc + 1) * GB * 2],
                            in_=ov[:].rearrange("p b c -> p (b c)"),
                        )
            elif "A" in phases:
                zo = cpool.tile([P, NB * 2], f32, tag="zo")
                nc.vector.memset(zo[:], 0.0)
                nc.sync.dma_start(out=out[:], in_=zo[:])
    nc.compile()
    return nc


# ---------------------------------------------------------------------------
_BUILT = None


def _get_built():
    global _BUILT
    if _BUILT is None:
        _BUILT = build_kernel()
    return _BUILT


def _host_prep(idx, knots, depth):
    depth = float(depth)
    ind = int(
        np.searchsorted(np.arange(1, D + 1, dtype=np.float64), depth, side="right")
    )
    ind = max(1, min(ind, D - 1))
    r = depth - float(ind)
    dcoord = (ind - 1) + r
    i0 = int(np.floor(dcoord))
    sd = dcoord - i0
    idp = np.clip(i0 - 1 + np.arange(4), 0, D - 1)
    powers = np.array([sd**3, sd**2, sd, 1.0], dtype=np.float64)
    wdv = (powers @ BASIS).astype(np.float32)
    wd_rep = np.tile(wdv[None, :], (P, 1))
    knots4 = knots[idp]  # [4, Z, Y, X, C] f32 view

    co = idx.astype(np.float64)
    iz = np.floor(co[:, 0]).astype(np.int64)
    iy = np.floor(co[:, 1]).astype(np.int64)
    ix = np.floor(co[:, 2]).astype(np.int64)
    sz = co[:, 0] - iz
    sy = co[:, 1] - iy
    sx = co[:, 2] - ix

    # x-window cardinal weights over 8 quad slots
    cx4 = (
        np.stack([sx**3, sx**2, sx, np.ones_like(sx)], 1) @ BASIS
    )  # [N, 4]
    q = ((ix - 1) & 3).astype(np.int64)
    cxw8 = np.zeros((N, 8), np.float64)
    np.put_along_axis(cxw8, q[:, None] + np.arange(4)[None, :], cx4, axis=1)
    cz4 = np.stack([sz**3, sz**2, sz, np.ones_like(sz)], 1) @ BASIS
    cy4 = np.stack([sy**3, sy**2, sy, np.ones_like(sy)], 1) @ BASIS
    wzy_all = (
        (cz4[:, :, None] * cy4[:, None, :]).reshape(N, 16).astype(np.float16)
    )
    wx_all = cxw8.astype(np.float16)

    in_maps = []
    unpack = []
    c0 = 1
    for core in range(NCORES):
        run = ZRUNS[core]
        sel = np.where((iz >= c0) & (iz < c0 + run))[0]
        n = len(sel)
        assert n <= NPC, (core, n)
        az = iz[sel] - c0
        row = ((az * NAY + (iy[sel] - 1)) * AXQ + ((ix[sel] - 1) >> 2)).astype(
            np.int64
        )
        order = np.argsort(row, kind="stable")
        sel = sel[order]
        rows_s = row[order]
        rows_pad = np.concatenate(
            [rows_s, np.full(NPC - n, rows_s[0] if n else 0, np.int64)]
        ).astype(np.int16)
        wx_pad = np.zeros((NPC, 8), np.float16)
        wx_pad[:n] = wx_all[sel]
        wzy_pad = np.zeros((NPC, 16), np.float16)
        wzy_pad[:n] = wzy_all[sel]

        # idxs: per 1024-call wrapped [16, 64], replicated to 128 partitions
        blk = (
            rows_pad.reshape(NG, 64, 16).transpose(0, 2, 1).reshape(NG, 16, 64)
        )  # [NG, 16, 64]
        idxs_core = np.tile(
            blk.transpose(1, 0, 2).reshape(16, NG * 64), (8, 1)
        )  # [128, NG*64]

        # weights: j = call*1024 + bl*128 + p -> [p, call*8 + bl]
        wx_core = (
            wx_pad.reshape(NG, GB, P, 8).transpose(2, 0, 1, 3).reshape(P, NB * 8)
        )
        wzy_core = (
            wzy_pad.reshape(NG, GB, P, 16).transpose(2, 0, 1, 3).reshape(P, NB * 16)
        )

        zs = c0 - 1
        kn = np.ascontiguousarray(
            knots4[:, zs : zs + ZW].transpose(0, 1, 2, 4, 3)
        ).astype(np.float16)  # [4, ZW, Y, C, X]

        in_maps.append(
            {
                "knots11": kn,
                "wd": wd_rep,
                "idxs16": np.ascontiguousarray(idxs_core),
                "wx8": np.ascontiguousarray(wx_core),
                "wzy16": np.ascontiguousarray(wzy_core),
            }
        )
        unpack.append((sel, n))
        c0 += run
    return in_maps, unpack


def kernel(idx, knots, depth):
    idx = np.asarray(idx, dtype=np.float32)
    knots = np.asarray(knots, dtype=np.float32)
    nc = _get_built()
    in_maps, unpack = _host_prep(idx, knots, depth)
    res = bass_utils.run_bass_kernel_spmd(nc, in_maps, core_ids=list(range(NCORES)))
    out_full = np.empty((N, 2), np.float32)
    for core in range(NCORES):
        sel, n = unpack[core]
        o = (
            res.results[core]["out"]
            .reshape(P, NG, GB, 2)
            .transpose(1, 2, 0, 3)
            .reshape(NPC, 2)
        )
        out_full[sel] = o[:n]
    return out_full


if __name__ == "__main__":
    nc = build_kernel()
    print("built ok")


# revision 28
# speedup vs baseline: 6.4211x; 3.1994x over previous
"""Catmull-Rom 4D spline interpolation kernel for Trainium2 (8 NeuronCores).

Problem: knots [16,64,128,128,2] f32, idx [262144,3] f32 (z,y,x coords),
depth scalar -> out [262144, 2] f32.

Strategy (v3, fp16 + dma_gather):
  - depth is a scalar -> D collapses host-side to 4 slabs + weights wd.
  - Points sharded by fixed z-cell runs ([8,8,8,8,7,7,7,7] over iz in
    [1,60]), so each core spans <= 8 z-cells -> its folded table has
    8*125*32 = 32000 quad-rows, indexable by int16 dma_gather indices.
  - Phase A (per core): depth-reduce the 11-slab fp16 z-window to
    V12[y,z,c,x], then lay out W3 rows (az, ay, axq) -> [c, kz, ky, ax4]
    = 128 fp16 = 256 B: the raw 4x4 z/y neighborhood per x-quad (cardinal
    basis -- pure copies from ky-shifted V12, no basis expansion; the
    spline weights ship from the host instead).
  - Phase B: one dma_gather descriptor per point reads 512 B (2 quad-rows
    = 8 ax slots covering the point's 4-ax window at quad offset q);
    multiply by host-shipped cardinal x-weights cxw8 (zeros outside the
    window) and wzy = wz4 (x) wy4 Catmull weights, then tree-reduce.
    The main multiplies run in DVE 2x fp16 mode (packed last axis).
"""
import sys

sys.path.insert(0, "/opt/trn_rl_repo")

import numpy as np

import concourse.mybir as mybir
import concourse.tile as tile_mod
from concourse import bass
from concourse.bacc import Bacc
from concourse.tile import TileContext
from concourse import bass_utils, library_config

# ---------------------------------------------------------------------------
# Workaround: this walrus build allows 1 sync wait per instruction (2 on
# InstEventSemaphore), but TileContext's tail drain carries one wait per DMA
# sem lane. Split the drain's waits onto EventSemaphore instructions.


def _patched_dab(self, tick_clock, wait_clock):
    nc = self.nc
    drain_bi = nc.sync.drain()
    wait_clock.add_sem_waits(
        drain_bi.ins, tile_mod.ScopedClock({None: tick_clock.global_clock})
    )
    si = drain_bi.ins.sync_info
    waits = list(si.on_wait) if si is not None else []
    if len(waits) > 1:
        si.on_wait = []
        bb = nc.cur_bb.bb
        insts = bb.instructions
        assert insts[-1].name == drain_bi.ins.name
        insts.pop()
        for i in range(0, len(waits), 2):
            ev = mybir.InstEventSemaphore(
                name=nc.get_next_instruction_name(), ins=[], outs=[]
            )
            ev.engine = drain_bi.ins.engine
            ev.sync_info = mybir.SyncInfo(on_wait=waits[i : i + 2], on_update=[])
            nc.register_instruction(ev)
            bb.add_instruction(ev)
        bb.add_instruction(drain_bi.ins)
    nc.all_engine_barrier()
    assert self.sems is not None
    popped = nc._tile_sem_poison_stack.pop()
    assert popped is self._sem_poison
    nc.clear_and_free_semaphores(list(self.sems.allocated().values()))
    nc.all_engine_barrier()


tile_mod.TileContext._drain_and_barrier = _patched_dab

# ---------------------------------------------------------------------------
D, Z, Y, X, C = 16, 64, 128, 128, 2
N = 262144
NCORES = 8
P = 128

ZRUNS = [8, 8, 8, 8, 7, 7, 7, 7]  # z-cells per core, covering iz in [1,60]
ZW = 11  # z-slab window per core (max run 8 + 3)
AZT = 8  # az table extent per core
NAY = 125  # ay in [0,124]
AXQ = 32  # x quads
NROWS = AZT * NAY * AXQ  # 32000 (+1 pad row)
NPC = 35840  # padded points per core
NB = NPC // P  # 280 blocks
GI = 1024  # idxs per dma_gather call (hw limit ~1024)
NG = NPC // GI  # 35 gather calls
GB = GI // P  # 8 blocks per gather
NCH = 7  # compute chunks
CB = NB // NCH  # 40 blocks per chunk
CG = NG // NCH  # 5 gathers per chunk

f32 = mybir.dt.float32
fp16 = mybir.dt.float16
i16 = mybir.dt.int16
AluOp = mybir.AluOpType

_HERMITE = np.array(
    [[2, -2, 1, 1], [-3, 3, -2, -1], [0, 0, 1, 0], [1, 0, 0, 0]], dtype=np.float64
)
_CR = np.array(
    [[0, 1, 0, 0], [0, 0, 1, 0], [-0.5, 0, 0.5, 0], [0, -0.5, 0, 0.5]],
    dtype=np.float64,
)
BASIS = _HERMITE @ _CR  # [4 powers (s^3..s^0), 4 knots]
BB = BASIS[::-1].copy()  # rows s^0..s^3 (jy/jz coefficient of each knot)


def build_kernel(reps=1, phases="AB"):
    """Per-core kernel (SPMD; per-core data differs). Inputs:
    knots11 [4, ZW, Y, C, X] fp16   host-sliced depth+z window (c before x)
    wd      [P, 4] f32              depth weights (replicated over partitions)
    idxs16  [128, NG*GI/16] i16     wrapped+replicated gather indices
    wb      [P, NB, 24] fp16        per-point cxw8 (8) + wzy (16)
    Output: out [P, NB*2] f32
    """
    # 32 KB descriptor carveout (2048 descs): lets two 1024-desc dma_gather
    # calls coexist in the SWDGE ring so gen(i+1) overlaps transfer(i)
    # instead of stalling on ring reclaim.
    nc = Bacc(
        "TRN2",
        target_bir_lowering=False,
        debug=False,
        num_devices=NCORES,
        dynamic_dma_scratch_size=32768,
    )
    knots11 = nc.dram_tensor("knots11", [4, ZW, Y, C, X], fp16, kind="ExternalInput")
    wd = nc.dram_tensor("wd", [P, 4], f32, kind="ExternalInput")
    idxs16 = nc.dram_tensor("idxs16", [128, NG * GI // 16], i16, kind="ExternalInput")
    wx8 = nc.dram_tensor("wx8", [P, NB * 8], fp16, kind="ExternalInput")
    wzy16 = nc.dram_tensor("wzy16", [P, NB * 16], fp16, kind="ExternalInput")
    out = nc.dram_tensor("out", [P, NB * 2], f32, kind="ExternalOutput")
    w3rows = nc.dram_tensor("w3rows", [NROWS + 1, 128], fp16, kind="Internal")

    with TileContext(nc) as tc:
      for _rep in range(reps):
        with tc.tile_pool(name="const", bufs=1) as cpool:
            wd_sb = cpool.tile([P, 4], f32)
            nc.sync.dma_start(out=wd_sb[:], in_=wd[:])
            idx_sb = cpool.tile([128, NG * GI // 16], i16)
            nc.sync.dma_start(out=idx_sb[:], in_=idxs16[:])
            wx_sb = cpool.tile([P, NB, 8], fp16)
            nc.sync.dma_start(out=wx_sb[:].rearrange("p b w -> p (b w)"), in_=wx8[:])
            wzy_sb = cpool.tile([P, NB, 16], fp16)
            nc.sync.dma_start(
                out=wzy_sb[:].rearrange("p b w -> p (b w)"), in_=wzy16[:]
            )

            if "A" in phases:
                # V12[y, z, c, x] fp16: depth-reduced window
                v12 = cpool.tile([P, ZW, C, X], fp16)
                with tc.tile_pool(name="pa", bufs=2) as pa:
                    for z0, zn in [(0, 4), (4, 4), (8, 3)]:
                        slabs = pa.tile([P, 4, zn, C * X], fp16, tag="slabs")
                        for d in range(4):
                            nc.sync.dma_start(
                                out=slabs[:, d, :, :],
                                in_=knots11[d, z0 : z0 + zn, :, :, :].rearrange(
                                    "z y c x -> y z (c x)"
                                ),
                            )
                        vsl = v12[:, z0 : z0 + zn, :, :].rearrange(
                            "p z c x -> p (z c x)"
                        )
                        nc.vector.tensor_scalar(
                            out=vsl,
                            in0=slabs[:, 0, :, :].rearrange("p z f -> p (z f)"),
                            scalar1=wd_sb[:, 0:1],
                            scalar2=None,
                            op0=AluOp.mult,
                        )
                        for d in range(1, 4):
                            nc.vector.scalar_tensor_tensor(
                                out=vsl,
                                in0=slabs[:, d, :, :].rearrange("p z f -> p (z f)"),
                                scalar=wd_sb[:, d : d + 1],
                                in1=vsl,
                                op0=AluOp.mult,
                                op1=AluOp.add,
                            )

                # ky-shifted copies of V12 (partition shifts via SBUF DMA)
                v12s = [v12]
                for ky in range(1, 4):
                    vk = cpool.tile([P, ZW, C, X], fp16, tag=f"v12s{ky}")
                    nc.sync.dma_start(
                        out=vk[0 : P - ky, :, :, :], in_=v12[ky:P, :, :, :]
                    )
                    v12s.append(vk)

                # W3 row (az, ay, axq) payload [c, kz, ky, ax4]: raw z/y
                # neighborhood values (cardinal basis; weights ship from host)
                with tc.tile_pool(name="pc", bufs=2) as pc:
                    for az in range(AZT):
                        w3t = pc.tile([P, AXQ, C, 4, 4, 4], fp16, tag="w3t")
                        for ky in range(4):
                            for c in range(C):
                                # <=3 free dims per ISA operand, shapes match
                                nc.vector.tensor_copy(
                                    out=w3t[0:NAY, :, c, :, ky, :],
                                    in_=v12s[ky][
                                        0:NAY, az : az + 4, c, :
                                    ].rearrange(
                                        "p kz (axq ax4) -> p axq kz ax4",
                                        axq=AXQ, ax4=4,
                                    ),
                                )
                        nc.sync.dma_start(
                            out=w3rows[az * NAY * AXQ : (az + 1) * NAY * AXQ, :]
                            .rearrange("(ay axq) f -> ay (axq f)", ay=NAY, axq=AXQ),
                            in_=w3t[0:NAY].rearrange(
                                "p axq c jz jy ax4 -> p (axq c jz jy ax4)"
                            ),
                        )
                # zero the pad row (read by idx NROWS-1 overlap)
                zt = cpool.tile([P, 128], fp16, tag="zt")
                nc.vector.memset(zt[0:1, :], 0.0)
                nc.sync.dma_start(out=w3rows[NROWS : NROWS + 1, :], in_=zt[0:1, :])

            if "B" in phases:
                nc.gpsimd.load_library(library_config.mlp)
                # per-gather tiles: each dma_gather owns its tile so the Pool
                # descriptor-gen of gather i+1 overlaps the DVE chain of
                # gather i (no whole-chunk tile to serialize on).
                with tc.tile_pool(name="pb", bufs=4) as pb:
                    for gc in range(NG):
                        g = pb.tile([P, GB, 256], fp16, tag="g")
                        nc.gpsimd.dma_gather(
                            out_ap=g[:],
                            in_ap=bass.AP(w3rows, 0, [[128, NROWS], [1, 256]]),
                            idxs_ap=idx_sb[
                                :, gc * (GI // 16) : (gc + 1) * (GI // 16)
                            ],
                            num_idxs=GI,
                            num_idxs_reg=GI,
                            elem_size=256,
                            elem_step=128,
                        )
                        # g: [p, (b r), m=(c kz ky), ax4] (<=3 free dims)
                        gv = g[:].rearrange(
                            "p b (r m ax) -> p (b r) m ax", r=2, m=32, ax=4
                        )
                        # p1: g *= cxw8 (bcast over m)
                        cxwb = (
                            wx_sb[:, gc * GB : (gc + 1) * GB, :]
                            .rearrange("p b (r i ax) -> p (b r) i ax", r=2, i=1, ax=4)
                            .to_broadcast([P, GB * 2, 32, 4])
                        )
                        nc.vector.tensor_tensor(
                            out=gv, in0=gv, in1=cxwb, op=AluOp.mult
                        )
                        # fold row2
                        g2 = g[:].rearrange("p b (r f) -> p b r f", r=2, f=128)
                        t = pb.tile([P, GB, 32, 4], fp16, tag="t")
                        nc.vector.tensor_tensor(
                            out=t[:].rearrange("p b m ax -> p b (m ax)"),
                            in0=g2[:, :, 0],
                            in1=g2[:, :, 1],
                            op=AluOp.add,
                        )
                        # fold ax4 4->2->1
                        u = pb.tile([P, GB, 32, 2], fp16, tag="u")
                        nc.vector.tensor_tensor(
                            out=u[:], in0=t[:, :, :, 0:2], in1=t[:, :, :, 2:4],
                            op=AluOp.add,
                        )
                        v = pb.tile([P, GB, 2, 16], fp16, tag="v")
                        vf = v[:].rearrange("p b c k -> p b (c k)")
                        nc.vector.tensor_tensor(
                            out=vf, in0=u[:, :, :, 0], in1=u[:, :, :, 1],
                            op=AluOp.add,
                        )
                        # *= wzy (bcast over c)
                        wzyb = (
                            wzy_sb[:, gc * GB : (gc + 1) * GB, :]
                            .rearrange("p b (i k) -> p b i k", i=1, k=16)
                            .to_broadcast([P, GB, 2, 16])
                        )
                        nc.vector.tensor_tensor(
                            out=v[:], in0=v[:], in1=wzyb, op=AluOp.mult
                        )
                        # reduce (kz, ky) -> f32
                        ov = pb.tile([P, GB, 2], f32, tag="ov")
                        nc.vector.tensor_reduce(
                            out=ov[:],
                            in_=v[:],
                            axis=mybir.AxisListType.X,
                            op=AluOp.add,
                        )
                        nc.sync.dma_start(
                            out=out[:, gc * GB * 2 : (gc + 1) * GB * 2],
                            in_=ov[:].rearrange("p b c -> p (b c)"),
                        )
            elif "A" in phases:
                zo = cpool.tile([P, NB * 2], f32, tag="zo")
                nc.vector.memset(zo[:], 0.0)
                nc.sync.dma_start(out=out[:], in_=zo[:])
    nc.compile()
    return nc


# ---------------------------------------------------------------------------
_BUILT = None


def _get_built():
    global _BUILT
    if _BUILT is None:
        _BUILT = build_kernel()
    return _BUILT


def _host_prep(idx, knots, depth):
    depth = float(depth)
    ind = int(
        np.searchsorted(np.arange(1, D + 1, dtype=np.float64), depth, side="right")
    )
    ind = max(1, min(ind, D - 1))
    r = depth - float(ind)
    dcoord = (ind - 1) + r
    i0 = int(np.floor(dcoord))
    sd = dcoord - i0
    idp = np.clip(i0 - 1 + np.arange(4), 0, D - 1)
    powers = np.array([sd**3, sd**2, sd, 1.0], dtype=np.float64)
    wdv = (powers @ BASIS).astype(np.float32)
    wd_rep = np.tile(wdv[None, :], (P, 1))
    knots4 = knots[idp]  # [4, Z, Y, X, C] f32 view

    co = idx.astype(np.float64)
    iz = np.floor(co[:, 0]).astype(np.int64)
    iy = np.floor(co[:, 1]).astype(np.int64)
    ix = np.floor(co[:, 2]).astype(np.int64)
    sz = co[:, 0] - iz
    sy = co[:, 1] - iy
    sx = co[:, 2] - ix

    # x-window cardinal weights over 8 quad slots
    cx4 = (
        np.stack([sx**3, sx**2, sx, np.ones_like(sx)], 1) @ BASIS
    )  # [N, 4]
    q = ((ix - 1) & 3).astype(np.int64)
    cxw8 = np.zeros((N, 8), np.float64)
    np.put_along_axis(cxw8, q[:, None] + np.arange(4)[None, :], cx4, axis=1)
    cz4 = np.stack([sz**3, sz**2, sz, np.ones_like(sz)], 1) @ BASIS
    cy4 = np.stack([sy**3, sy**2, sy, np.ones_like(sy)], 1) @ BASIS
    wzy_all = (
        (cz4[:, :, None] * cy4[:, None, :]).reshape(N, 16).astype(np.float16)
    )
    wx_all = cxw8.astype(np.float16)

    in_maps = []
    unpack = []
    c0 = 1
    for core in range(NCORES):
        run = ZRUNS[core]
        sel = np.where((iz >= c0) & (iz < c0 + run))[0]
        n = len(sel)
        assert n <= NPC, (core, n)
        az = iz[sel] - c0
        row = ((az * NAY + (iy[sel] - 1)) * AXQ + ((ix[sel] - 1) >> 2)).astype(
            np.int64
        )
        order = np.argsort(row, kind="stable")
        sel = sel[order]
        rows_s = row[order]
        rows_pad = np.concatenate(
            [rows_s, np.full(NPC - n, rows_s[0] if n else 0, np.int64)]
        ).astype(np.int16)
        wx_pad = np.zeros((NPC, 8), np.float16)
        wx_pad[:n] = wx_all[sel]
        wzy_pad = np.zeros((NPC, 16), np.float16)
        wzy_pad[:n] = wzy_all[sel]

        # idxs: per 1024-call wrapped [16, 64], replicated to 128 partitions
        blk = (
            rows_pad.reshape(NG, 64, 16).transpose(0, 2, 1).reshape(NG, 16, 64)
        )  # [NG, 16, 64]
        idxs_core = np.tile(
            blk.transpose(1, 0, 2).reshape(16, NG * 64), (8, 1)
        )  # [128, NG*64]

        # weights: j = call*1024 + bl*128 + p -> [p, call*8 + bl]
        wx_core = (
            wx_pad.reshape(NG, GB, P, 8).transpose(2, 0, 1, 3).reshape(P, NB * 8)
        )
        wzy_core = (
            wzy_pad.reshape(NG, GB, P, 16).transpose(2, 0, 1, 3).reshape(P, NB * 16)
        )

        zs = c0 - 1
        kn = np.ascontiguousarray(
            knots4[:, zs : zs + ZW].transpose(0, 1, 2, 4, 3)
        ).astype(np.float16)  # [4, ZW, Y, C, X]

        in_maps.append(
            {
                "knots11": kn,
                "wd": wd_rep,
                "idxs16": np.ascontiguousarray(idxs_core),
                "wx8": np.ascontiguousarray(wx_core),
                "wzy16": np.ascontiguousarray(wzy_core),
            }
        )
        unpack.append((sel, n))
        c0 += run
    return in_maps, unpack


def kernel(idx, knots, depth):
    idx = np.asarray(idx, dtype=np.float32)
    knots = np.asarray(knots, dtype=np.float32)
    nc = _get_built()
    in_maps, unpack = _host_prep(idx, knots, depth)
    res = bass_utils.run_bass_kernel_spmd(nc, in_maps, core_ids=list(range(NCORES)))
    out_full = np.empty((N, 2), np.float32)
    for core in range(NCORES):
        sel, n = unpack[core]
        o = (
            res.results[core]["out"]
            .reshape(P, NG, GB, 2)
            .transpose(1, 2, 0, 3)
            .reshape(NPC, 2)
        )
        out_full[sel] = o[:n]
    return out_full


if __name__ == "__main__":
    nc = build_kernel()
    print("built ok")
